# revision 13
# baseline (speedup 1.0000x reference)
"""CASSI forward kernel for Trainium2, SPMD across 8 NeuronCores.

Computation (per batch sample b):
    y2[i, c]     = sum_l x[l, i, c-2l] * phi[i, c-2l]         (scatter-accumulate)
    out[l, i, j] = y2[i, 2l+j] * phi[i, j]                    (windowed gather)

Sharding: data-parallel over batch (B=8 -> one sample per core), phi replicated.

Design (measured DMA roofline on this part: loads-only 347 GB/s,
stores-only 357 GB/s, packet-interleaved mixed R/W only 319 GB/s):
  - bf16 wire format (correctness gate is rel_err < 2e-2; bf16 wire with
    f32 PSUM accumulation lands ~3.5e-3 for the binary phi of the
    reference, ~7e-3 for a continuous phi), halving the 59MB/core f32
    memory floor to 29.4MB/core.
  - Host packs x into a per-partition-contiguous flat layout
    ([p][block][band][col]) so every DMA is a clean contiguous run;
    host unpacks + upcasts the flat bf16 output.
  - x loads AND out stores share the SP HWDGE queue: HWDGE executes
    in FIFO order per queue, so reads and writes phase-separate at
    block granularity instead of interleaving per packet (HBM bus
    turnaround costs ~10% of mixed-traffic bandwidth: the phased queue
    runs ~83.5us of DMA vs ~92us interleaved).
  - Per 128-row block: DVE does stage-1 x*phi (bf16 2x mode) and the
    stage-2 windowed multiplies; ACT does the PSUM->SBUF y2 copy
    (f32->bf16); PE scatter-accumulates bands into PSUM via identity
    matmuls (f32, exact); constant phi/eye tiles are loaded once,
    outside the timed loop.

Measured on silicon (For_i-1001 marginal, device-resident inputs):
~87-90us per pass in quiet periods, ~96-98us under device contention
epochs; vs a ~84us phased-DMA queue-time floor for this traffic.
Rejected alternatives (measured): GPSIMD stage-1 mults (+23us), 14-band
load slabs (+8us), stage-2 store granularity 4/14/28 (+3..6us),
deferring all stage-2 past the loads (+2us), staggered-reset back-edge
(+0 in noise), 16-deep load buffering (+0 in noise).
"""

import sys

if "/opt/trn_rl_repo" not in sys.path:
    sys.path.insert(0, "/opt/trn_rl_repo")

import numpy as np
import ml_dtypes

import concourse.bass as bass
import concourse.bacc as bacc
import concourse.mybir as mybir
import concourse.tile as tile
from concourse.bass_utils import run_bass_kernel_spmd

BF16 = np.dtype(ml_dtypes.bfloat16)

B = 8
L, M, N, S = 28, 512, 512, 2
NOUT = N + S * (L - 1)  # 566
P = 128
NBLK = M // P  # 4 row blocks
NH = 4  # band-dim slabs per block
HB = L // NH  # 7 bands per slab
XW = NBLK * L * N  # flat per-partition elements of x / out

_cached = {}

# Tuning knobs (read at build time).
MULT_ENG = lambda b, h: False  # False -> DVE, True -> GPSIMD(Pool)
S2_GRAN = 7  # bands per stage-2 instruction / store
HB_ = 7  # bands per load slab (must divide L)
MULT_PIECE = 7  # bands per stage-1 multiply instruction
ST_SYNC = True  # stores on the SP queue (True) vs Activation queue (False)
S2_DEFER_ALL = False  # emit all stage-2 after all loads (pure phases)
COPY_SCALAR = True  # PSUM->SBUF y2 copy on the Activation engine (else DVE)
X_BUFS = 10
O_BUFS = 6
STAGGER = False  # staggered-reset back-edge barrier on For_i


def _body_bf16(nc, tc, x_d, out_d, phi_sb, eye_sb, zero_sb, pools):
    bf16 = mybir.dt.bfloat16
    f32 = mybir.dt.float32
    y_pool, ysb_pool, x_pool, o_pool = pools
    st_eng = nc.sync if ST_SYNC else nc.scalar

    def emit_stage2(b, y2, phi_blk):
        l0 = 0
        while l0 < L:
            g = min(S2_GRAN, L - l0)
            ot = o_pool.tile([P, g * N], bf16, tag="ot")
            o3 = ot[:, 0 : g * N].rearrange("p (l n) -> p l n", n=N)
            # windowed view: band j reads y2[:, 2*(l0+j) : 2*(l0+j)+512]
            base = y2[:, S * l0 : S * l0 + N].unsqueeze(1)
            win = bass.AP(
                base.tensor,
                base.offset,
                [list(base.ap[0]), [S, g], list(base.ap[2])],
            )
            phi_g = phi_blk.unsqueeze(1).broadcast_to([P, g, N])
            nc.vector.tensor_tensor(o3, win, phi_g, mybir.AluOpType.mult)
            st_eng.dma_start(
                out_d[:, (b * L + l0) * N : (b * L + l0 + g) * N], ot[:, :]
            )
            l0 += g

    pending = None

    for b in range(NBLK):
        phi_blk = phi_sb[:, b * N : (b + 1) * N]

        y2p = y_pool.tile([P, 1024], f32, tag="y2p")
        # Arm PSUM bank1 (cols 512..566): first writer must be start=True
        # over the full eventually-accumulated region.
        nc.tensor.matmul(
            y2p[:, N:NOUT], eye_sb[:, :], zero_sb[:, :], start=True, stop=False
        )

        for l0 in range(0, L, HB_):
            hb = min(HB_, L - l0)
            xt = x_pool.tile([P, hb * N], bf16, tag="xt")
            nc.sync.dma_start(
                xt[:, :], x_d[:, (b * L + l0) * N : (b * L + l0 + hb) * N]
            )
            mult_eng = nc.gpsimd if MULT_ENG(b, l0) else nc.vector
            for m0 in range(0, hb, MULT_PIECE):
                mw = min(MULT_PIECE, hb - m0)
                xs = xt[:, m0 * N : (m0 + mw) * N].rearrange(
                    "p (l n) -> p l n", n=N
                )
                phi_m = phi_blk.unsqueeze(1).broadcast_to([P, mw, N])
                mult_eng.tensor_tensor(xs, xs, phi_m, mybir.AluOpType.mult)
            # scatter-accumulate into PSUM on PE; bands cross the 512-wide
            # bank boundary, so split each into <=2 matmuls
            for j in range(hb):
                l = l0 + j
                w0 = N - S * l
                nc.tensor.matmul(
                    y2p[:, S * l : N],
                    eye_sb[:, :],
                    xt[:, j * N : j * N + w0],
                    start=(l == 0),
                    stop=(l == L - 1),
                )
                if l > 0:
                    nc.tensor.matmul(
                        y2p[:, N : N + S * l],
                        eye_sb[:, :],
                        xt[:, j * N + w0 : (j + 1) * N],
                        start=False,
                        stop=(l == L - 1),
                    )

        y2 = ysb_pool.tile([P, NOUT], bf16, tag="y2")
        if COPY_SCALAR:
            nc.scalar.copy(y2[:, :], y2p[:, 0:NOUT])
        else:
            nc.vector.tensor_copy(y2[:, :], y2p[:, 0:NOUT])

        # Stage-2 of block b-1 is emitted AFTER block b's matmuls: the Tile
        # scheduler's priority heap follows emission order, keeping the
        # load queue fed.
        if S2_DEFER_ALL:
            pending = (pending or []) + [(b, y2, phi_blk)]
        else:
            if pending is not None:
                emit_stage2(*pending)
            pending = (b, y2, phi_blk)

    if S2_DEFER_ALL:
        for args in pending:
            emit_stage2(*args)
    else:
        emit_stage2(*pending)


def _build_nc(loop: int = 1):
    nc = bacc.Bacc("TRN2", target_bir_lowering=False, debug=False)
    bf16 = mybir.dt.bfloat16
    x_d = nc.dram_tensor("x", [P, XW], bf16, kind="ExternalInput").ap()
    phi_d = nc.dram_tensor("phi", [P, NBLK * N], bf16, kind="ExternalInput").ap()
    eye_d = nc.dram_tensor("eye", [P, P], bf16, kind="ExternalInput").ap()
    out_d = nc.dram_tensor("out", [P, XW], bf16, kind="ExternalOutput").ap()

    with tile.TileContext(nc) as tc:
        with (
            tc.tile_pool(name="phip", bufs=1) as phi_pool,
            tc.tile_pool(name="ypsum", bufs=2, space="PSUM") as y_pool,
            tc.tile_pool(name="ysb", bufs=NBLK) as ysb_pool,
            tc.tile_pool(name="xp", bufs=X_BUFS) as x_pool,
            tc.tile_pool(name="op", bufs=O_BUFS) as o_pool,
        ):
            # Constants: loaded once, on the (otherwise idle) Activation
            # queue, outside the timed loop body.
            phi_sb = phi_pool.tile([P, NBLK * N], bf16)
            nc.scalar.dma_start(phi_sb[:, :], phi_d)
            eye_sb = phi_pool.tile([P, P], bf16)
            nc.scalar.dma_start(eye_sb[:, :], eye_d)
            zero_sb = phi_pool.tile([P, S * (L - 1)], bf16)
            nc.vector.memset(zero_sb[:, :], 0.0)

            pools = (y_pool, ysb_pool, x_pool, o_pool)

            def emit():
                _body_bf16(nc, tc, x_d, out_d, phi_sb, eye_sb, zero_sb, pools)

            if loop == 1:
                emit()
            elif loop < 0:
                with tc.For_i(0, -loop, 1, staggered_reset=STAGGER):
                    emit()
            else:
                for _ in range(loop):
                    emit()

    nc.compile()
    return nc


def _get_nc():
    if "nc" not in _cached:
        _cached["nc"] = _build_nc()
    return _cached["nc"]


def _pack_x(x_core: np.ndarray) -> np.ndarray:
    """(L, M, N) f32 -> [P, XW] bf16 with [p, ((blk*L + l)*N + n)] layout."""
    v = x_core.reshape(L, NBLK, P, N).transpose(2, 1, 0, 3)
    return np.ascontiguousarray(v).astype(BF16).reshape(P, XW)


def _pack_phi(phi: np.ndarray) -> np.ndarray:
    v = phi.reshape(NBLK, P, N).transpose(1, 0, 2)
    return np.ascontiguousarray(v).astype(BF16).reshape(P, NBLK * N)


def _unpack_out(o_core: np.ndarray) -> np.ndarray:
    """[P, XW] bf16 -> (L, M, N) f32."""
    v = o_core.reshape(P, NBLK, L, N).transpose(2, 1, 0, 3)
    return np.ascontiguousarray(v).astype(np.float32).reshape(L, M, N)


def kernel(x: np.ndarray, phi: np.ndarray) -> np.ndarray:
    assert x.shape == (B, L, M, N) and phi.shape == (M, N)
    nc = _get_nc()
    x = np.asarray(x, dtype=np.float32)
    phi_p = _pack_phi(np.asarray(phi, dtype=np.float32))
    eye = np.eye(P, dtype=np.float32).astype(BF16)
    in_maps = [{"x": _pack_x(x[i]), "phi": phi_p, "eye": eye} for i in range(B)]
    res = run_bass_kernel_spmd(nc, in_maps, list(range(B)))
    return np.stack([_unpack_out(r["out"]) for r in res.results], axis=0)


if __name__ == "__main__":
    x = np.random.randn(B, L, M, N).astype(np.float32)
    phi = (np.random.randn(M, N) > 0).astype(np.float32)
    out = kernel(x, phi)
    print("out", out.shape, out.dtype)
